# revision 7
# baseline (speedup 1.0000x reference)
"""Distributed KNN retrieval (Database topk=4) on 8 Trainium2 NeuronCores.

Pipeline (per core, SPMD over 8 cores; corpus sharded along N):
  1. L1-normalize queries on device (fp32), cast to bf16, PE-transpose.
  2. Scan the core's 50000-column shard in 1024-column chunks:
     bf16 matmul -> PSUM fp32 sims -> DVE max8 + max_index per chunk
     (candidate values + local indices).
  3. Level-2: max8 + max_index over the 49*8 chunk candidates -> per-core
     top-8 candidate positions; 2-hop indirect-DMA gather resolves the
     winner corpus indices and fetches their fp32 embedding rows.
  4. Exact fp32 rescore (DVE mul + group reduce) of the 8 candidates.
Host merges 8 cores x 8 exact-scored candidates -> global top-4.

The masked range [start, end) is handled by zeroing those columns in the
bf16 shard: masked sims become exactly 0 and can never reach the per-core
top-8 (all true top sims are strictly positive for any realistic corpus),
while the fp32 rescore table keeps original values for exact outputs.
"""

import os

import numpy as np
import ml_dtypes

import concourse.bass as bass
import concourse.bacc as bacc
import concourse.mybir as mybir
import concourse.tile as tile
import concourse.bass_utils as bass_utils
from concourse.masks import make_identity

Q, D, N, TOPK = 256, 768, 400000, 4
NCORES = 8
NSHARD = N // NCORES          # 50000
CHUNK = 1024
NCH = (NSHARD + CHUNK - 1) // CHUNK   # 49
NPAD = NCH * CHUNK            # 50176
KT = D // 128                 # 6 k-tiles
MT = Q // 128                 # 2 m-tiles
CAND = NCH * 8                # 392 level-1 candidates per core per query
L2K = 8                       # candidates rescored per core per query

_prog_cache = {}


def _install_ntff_hook_shim():
    """Provide antenv.axon_hooks (absent in this image) so that
    run_bass_kernel_spmd(trace=True) can capture NTFF profiles through the
    injected libaxon_pjrt.so. Mirrors trn_agent_boot/trn_boot.py."""
    import sys
    import types
    import ctypes
    import contextlib

    if "antenv.axon_hooks" in sys.modules:
        return
    mod = types.ModuleType("antenv.axon_hooks")
    state = {"hook": None}
    mod.set_axon_ntff_profile_hook = lambda h: state.__setitem__("hook", h)
    mod.get_axon_ntff_profile_hook = lambda: state["hook"]
    sys.modules["antenv.axon_hooks"] = mod

    so_path = "/opt/axon/libaxon_pjrt.so"
    if not os.path.exists(so_path):
        return
    try:
        lib = ctypes.CDLL(so_path)
    except OSError:
        return
    if not hasattr(lib, "axon_start_nrt_profile"):
        return
    lib.axon_start_nrt_profile.argtypes = [ctypes.POINTER(ctypes.c_int64),
                                           ctypes.c_size_t]
    lib.axon_start_nrt_profile.restype = ctypes.c_int64
    lib.axon_stop_nrt_profile.argtypes = [ctypes.c_char_p]
    lib.axon_stop_nrt_profile.restype = ctypes.c_int64

    @contextlib.contextmanager
    def _hook(output_dir, device_ids):
        import jax
        jax.devices()
        if device_ids:
            ids = (ctypes.c_int64 * len(device_ids))(*device_ids)
            rc = lib.axon_start_nrt_profile(ids, len(device_ids))
        else:
            rc = lib.axon_start_nrt_profile(None, 0)
        if rc != 0:
            raise RuntimeError(f"axon_start_nrt_profile rc={rc}")
        try:
            yield
        finally:
            n = lib.axon_stop_nrt_profile(str(output_dir).encode())
            print(f"ntff profile: {n} file(s) written to {output_dir}")

    mod.set_axon_ntff_profile_hook(_hook)


def _build_program(debug_taps=False):
    nc = bacc.Bacc(None, target_bir_lowering=False, debug=False)

    q_dram = nc.dram_tensor("q", [Q, D], mybir.dt.float32, kind="ExternalInput")
    # emb shard, bf16, host-packed layout [chunk, partition, ktile*CHUNK]:
    # embL[j, p, t*CHUNK + n] = emb_bf16[t*128 + p, j*CHUNK + n]
    embL = nc.dram_tensor("embL", [NCH, 128, KT * CHUNK], mybir.dt.bfloat16,
                          kind="ExternalInput")
    # fp32 shard transposed (rows = corpus columns) for the exact rescore
    embT = nc.dram_tensor("embT", [NSHARD, D], mybir.dt.float32,
                          kind="ExternalInput")

    out_vals = nc.dram_tensor("out_vals", [Q, L2K], mybir.dt.float32,
                              kind="ExternalOutput")
    out_ids = nc.dram_tensor("out_ids", [Q, L2K], mybir.dt.uint32,
                             kind="ExternalOutput")
    if debug_taps:
        dbg_qn = nc.dram_tensor("dbg_qn", [Q, D], mybir.dt.float32,
                                kind="ExternalOutput")
        dbg_qt = nc.dram_tensor("dbg_qt", [128, KT * 128 * MT], mybir.dt.bfloat16,
                                kind="ExternalOutput")
        dbg_va = nc.dram_tensor("dbg_va", [Q, CAND], mybir.dt.float32,
                                kind="ExternalOutput")
        dbg_ia = nc.dram_tensor("dbg_ia", [Q, CAND], mybir.dt.uint32,
                                kind="ExternalOutput")
        dbg_p8 = nc.dram_tensor("dbg_p8", [Q, L2K], mybir.dt.uint32,
                                kind="ExternalOutput")
        dbg_l2v = nc.dram_tensor("dbg_l2v", [Q, L2K], mybir.dt.float32,
                                 kind="ExternalOutput")
        dbg_sim0 = nc.dram_tensor("dbg_sim0", [Q, CHUNK], mybir.dt.float32,
                                  kind="ExternalOutput")

    with tile.TileContext(nc) as tc:
        with tc.tile_pool(name="persist", bufs=1) as pp:
            qn = [pp.tile([128, D], mybir.dt.float32, tag=f"qn{m}", name=f"qn{m}")
                  for m in range(MT)]
            qT = pp.tile([128, KT, 128 * MT], mybir.dt.bfloat16, tag="qT")
            vals_all = [pp.tile([128, CAND], mybir.dt.float32, tag=f"va{m}", name=f"va{m}")
                        for m in range(MT)]
            ids_all = [pp.tile([128, CAND], mybir.dt.uint32, tag=f"ia{m}", name=f"ia{m}")
                       for m in range(MT)]
            base_full = pp.tile([128, CAND], mybir.dt.uint32, tag="base")
            qid = pp.tile([128, 1], mybir.dt.uint32, tag="qid")

            # chunk base offsets, identical in every partition
            nc.gpsimd.iota(base_full[:].rearrange("p (c k) -> p c k", k=8),
                           pattern=[[CHUNK, NCH], [0, 8]], base=0,
                           channel_multiplier=0)
            # partition index (query id within m-tile)
            nc.gpsimd.iota(qid[:], pattern=[[0, 1]], base=0,
                           channel_multiplier=1)

            # ---------- query prep ----------
            with (
                tc.tile_pool(name="prep_sb", bufs=2) as sp,
                tc.tile_pool(name="prep_ps", bufs=2, space="PSUM") as pps,
            ):
                ident = sp.tile([128, 128], mybir.dt.bfloat16, tag="ident")
                make_identity(nc, ident[:])
                for m in range(MT):
                    q_sb = sp.tile([128, D], mybir.dt.float32, tag="qsb")
                    nc.sync.dma_start(q_sb[:], q_dram.ap()[m * 128:(m + 1) * 128, :])
                    ssum = sp.tile([128, 1], mybir.dt.float32, tag="ssum")
                    nc.vector.tensor_reduce(ssum[:], q_sb[:],
                                            axis=mybir.AxisListType.X,
                                            op=mybir.AluOpType.add,
                                            apply_absolute_value=True)
                    nc.vector.tensor_scalar_max(ssum[:], ssum[:], 1e-12)
                    rcp = sp.tile([128, 1], mybir.dt.float32, tag="rcp")
                    nc.vector.reciprocal(rcp[:], ssum[:])
                    nc.scalar.mul(qn[m][:], q_sb[:], rcp[:])
                    qn_bf = sp.tile([128, D], mybir.dt.bfloat16, tag="qnbf")
                    nc.vector.tensor_copy(qn_bf[:], qn[m][:])
                    for t in range(KT):
                        tp = pps.tile([128, 128], mybir.dt.bfloat16, tag="tp")
                        nc.tensor.transpose(tp[:], qn_bf[:, t * 128:(t + 1) * 128],
                                            ident[:])
                        nc.vector.tensor_copy(
                            qT[:, t, m * 128:(m + 1) * 128], tp[:])
                    if debug_taps:
                        nc.sync.dma_start(
                            dbg_qn.ap()[m * 128:(m + 1) * 128, :], qn[m][:])
                if debug_taps:
                    nc.sync.dma_start(
                        dbg_qt.ap()[:],
                        qT[:].rearrange("p t x -> p (t x)"))

            # ---------- phase 1: scan shard ----------
            with (
                tc.tile_pool(name="rhs_sb", bufs=4) as rp,
                tc.tile_pool(name="sim_ps", bufs=4, space="PSUM") as sps,
            ):
                for j in range(NCH):
                    rhs = rp.tile([128, KT, CHUNK], mybir.dt.bfloat16, tag="rhs")
                    nc.sync.dma_start(rhs[:], embL.ap()[j].rearrange(
                        "p (t n) -> p t n", t=KT))
                    for m in range(MT):
                        psum = sps.tile([128, CHUNK], mybir.dt.float32, tag="sim")
                        for t in range(KT):
                            for h in range(CHUNK // 512):
                                nc.tensor.matmul(
                                    psum[:, h * 512:(h + 1) * 512],
                                    qT[:, t, m * 128:(m + 1) * 128],
                                    rhs[:, t, h * 512:(h + 1) * 512],
                                    start=(t == 0), stop=(t == KT - 1))
                        if debug_taps and j == 0:
                            s0 = rp.tile([128, CHUNK], mybir.dt.float32,
                                         tag="s0", name=f"s0_{m}")
                            nc.vector.tensor_copy(s0[:], psum[:])
                            nc.sync.dma_start(
                                dbg_sim0.ap()[m * 128:(m + 1) * 128, :], s0[:])
                        vs = vals_all[m][:, j * 8:(j + 1) * 8]
                        nc.vector.max(vs, psum[:])
                        nc.vector.max_index(ids_all[m][:, j * 8:(j + 1) * 8],
                                            vs, psum[:])

            # ---------- level 2 + gather + exact rescore ----------
            with (
                tc.tile_pool(name="l2_sb", bufs=2) as l2p,
                tc.tile_pool(name="l2_dram", bufs=2, space="DRAM") as dp,
            ):
                for m in range(MT):
                    # local -> within-shard indices
                    nc.vector.tensor_tensor(ids_all[m][:], ids_all[m][:],
                                            base_full[:],
                                            op=mybir.AluOpType.add)
                    l2v = l2p.tile([128, L2K], mybir.dt.float32, tag="l2v")
                    p8 = l2p.tile([128, L2K], mybir.dt.uint32, tag="p8")
                    nc.vector.max(l2v[:], vals_all[m][:])
                    nc.vector.max_index(p8[:], l2v[:], vals_all[m][:])
                    if debug_taps:
                        nc.sync.dma_start(
                            dbg_va.ap()[m * 128:(m + 1) * 128, :], vals_all[m][:])
                        nc.sync.dma_start(
                            dbg_ia.ap()[m * 128:(m + 1) * 128, :], ids_all[m][:])
                        nc.sync.dma_start(
                            dbg_p8.ap()[m * 128:(m + 1) * 128, :], p8[:])
                        nc.sync.dma_start(
                            dbg_l2v.ap()[m * 128:(m + 1) * 128, :], l2v[:])

                    # flat offsets into this m-tile's [128, CAND] id table
                    off = l2p.tile([128, L2K], mybir.dt.uint32, tag="off")
                    qsc = l2p.tile([128, 1], mybir.dt.uint32, tag="qsc")
                    nc.vector.tensor_scalar_mul(qsc[:], qid[:], float(CAND))
                    nc.vector.tensor_tensor(off[:], p8[:],
                                            qsc[:].to_broadcast([128, L2K]),
                                            op=mybir.AluOpType.add)

                    ids_dram = dp.tile([128, CAND], mybir.dt.uint32, name=f"idsd{m}")
                    nc.sync.dma_start(ids_dram[:], ids_all[m][:])
                    ids_win = l2p.tile([128, L2K], mybir.dt.uint32, tag="idswin")
                    for r in range(L2K):
                        nc.gpsimd.indirect_dma_start(
                            out=ids_win[:, r:r + 1], out_offset=None,
                            in_=ids_dram[:].rearrange("p f -> (p f)").unsqueeze(1),
                            in_offset=bass.IndirectOffsetOnAxis(
                                ap=off[:, r:r + 1], axis=0))
                    nc.sync.dma_start(
                        out_ids.ap()[m * 128:(m + 1) * 128, :], ids_win[:])

                    # guard against the (impossible in practice) case of a
                    # zero-pad column winning: clamp into the shard
                    nc.vector.tensor_scalar_min(ids_win[:], ids_win[:],
                                                float(NSHARD - 1))

                    cand = l2p.tile([128, L2K, D], mybir.dt.float32, tag="cand")
                    for r in range(L2K):
                        nc.gpsimd.indirect_dma_start(
                            out=cand[:, r, :], out_offset=None,
                            in_=embT.ap()[:],
                            in_offset=bass.IndirectOffsetOnAxis(
                                ap=ids_win[:, r:r + 1], axis=0))

                    nc.vector.tensor_tensor(
                        cand[:], cand[:],
                        qn[m][:].unsqueeze(1).to_broadcast([128, L2K, D]),
                        op=mybir.AluOpType.mult)
                    resc = l2p.tile([128, L2K], mybir.dt.float32, tag="resc")
                    nc.vector.tensor_reduce(resc[:].unsqueeze(2), cand[:],
                                            axis=mybir.AxisListType.X,
                                            op=mybir.AluOpType.add)
                    nc.sync.dma_start(
                        out_vals.ap()[m * 128:(m + 1) * 128, :], resc[:])

    nc.compile()
    return nc


def _get_program():
    if "nc" not in _prog_cache:
        _prog_cache["nc"] = _build_program()
    return _prog_cache["nc"]


def _prepare_core_inputs(q, emb, start, end):
    """Shard + pack inputs for each core. Returns list of per-core dicts."""
    emb_bf = emb.astype(ml_dtypes.bfloat16)
    if end > start:
        emb_bf[:, start:end] = 0
    in_maps = []
    for c in range(NCORES):
        lo = c * NSHARD
        shard_bf = emb_bf[:, lo:lo + NSHARD]
        pad = np.zeros((D, NPAD), dtype=ml_dtypes.bfloat16)
        pad[:, :NSHARD] = shard_bf
        embL = np.ascontiguousarray(
            pad.reshape(KT, 128, NCH, CHUNK).transpose(2, 1, 0, 3)
        ).reshape(NCH, 128, KT * CHUNK)
        embT = np.ascontiguousarray(emb[:, lo:lo + NSHARD].T)
        in_maps.append({"q": np.ascontiguousarray(q, dtype=np.float32),
                        "embL": embL, "embT": embT})
    return in_maps


def kernel(query, embeddings, start, end):
    q = np.asarray(query, dtype=np.float32)
    emb = np.asarray(embeddings, dtype=np.float32)
    start_i = int(np.asarray(start))
    end_i = int(np.asarray(end))
    assert q.shape == (Q, D) and emb.shape == (D, N)

    nc = _get_program()
    in_maps = _prepare_core_inputs(q, emb, start_i, end_i)

    trace = os.environ.get("KNN_TRACE", "0") == "1"
    if trace:
        _install_ntff_hook_shim()
    res = bass_utils.run_bass_kernel_spmd(
        nc, in_maps, core_ids=list(range(NCORES)), trace=trace)
    if trace:
        _prog_cache["last_exec_time_ns"] = res.exec_time_ns
        _prog_cache["last_results"] = res

    vals = np.stack([r["out_vals"] for r in res.results])          # [8, Q, 8]
    ids = np.stack([r["out_ids"] for r in res.results]).astype(np.int64)
    np.clip(ids, 0, NSHARD - 1, out=ids)
    gids = ids + (np.arange(NCORES, dtype=np.int64) * NSHARD)[:, None, None]

    allv = vals.transpose(1, 0, 2).reshape(Q, NCORES * L2K)
    allg = gids.transpose(1, 0, 2).reshape(Q, NCORES * L2K)
    # top-4 by value desc, index asc on ties (jax.lax.top_k tie rule)
    order = np.lexsort((allg, -allv), axis=1)[:, :TOPK]
    top_v = np.take_along_axis(allv, order, axis=1).astype(np.float32)
    top_i = np.take_along_axis(allg, order, axis=1).astype(np.int32)
    return top_v, top_i
